# revision 20
# baseline (speedup 1.0000x reference)
"""Chamfer-like distance loss on Trainium2 (Bass/Tile), 8-core SPMD.

Problem: depth_pred (4,1,64,64), boundary_gt (4,1,64,64).
  g = sqrt(sobel_x(depth)^2 + sobel_y(depth)^2 + 1e-8)  flattened to (B, N=4096)
  b = boundary flattened (B, 4096)
  d[i,j] = |g_i - b_j|;  out = mean_i min_j d  +  mean_j min_i d

Sharding: core k handles batch k//2, image-row half k%2 (32 rows = 2048 i's).
Each core computes, for its 2048 gradient points vs all 4096 boundary points:
  - rowmin: min_j |g_i - b_j| for each of its i  -> summed into rowsum (128,1)
  - colmin partial: min over its i of |g_i - b_j| for every j -> colmin (128,32)
Host combines: dist1 = sum of all rowsums / 16384; per batch the two cores'
colmin partials are elementwise-min'd, then dist2 = sum / 16384.

On-device layout:
  - Sobel runs in transposed layout: image cols on partitions (64), rows on
    the free axis, so the per-i-tile scalar vector g_s (128,16) is formed with
    two plain copies (no transpose needed).
  - Main loop, i-tile t of 16: partitions = 128 gradient points, free = all
    4096 boundary points. ACT produces d = Abs(-1*b_bcast + g) in one op
    (per-partition bias), DVE updates the running colmin (tensor_tensor min)
    and reduces the rowmin (tensor_reduce min / fused tensor_scalar accum).
  - colmin accumulators are partition-junk (partitions = i-lane); the tail
    PE-transposes 128x128 blocks into PSUM and free-reduces them. Two
    accumulators (tiles 0-7, 8-15) let the first tail overlap the second
    half of the main loop.
"""

import os
import sys

import numpy as np

for _p in ("/opt/trn_rl_repo", os.path.expanduser("~/.axon_site/_ro/trn_rl_repo")):
    if os.path.isdir(_p) and _p not in sys.path:
        sys.path.insert(0, _p)

import concourse.bass as bass
import concourse.bacc as bacc
import concourse.tile as tile
from concourse import mybir
from concourse.bass_utils import run_bass_kernel_spmd
from concourse import dve_ops
from concourse.dve_spec import Spec, Src0, C0, C1, maxx, minn, lower, _has_src1
from concourse.dve_uop import DveOpSpec


def _register_absdiff_min_op():
    """Custom DVE op: out = |in0 - s0|, accum_out = min(s1, min_k out).
    Fuses the abs-diff production and the free-axis min reduce into one
    1 elem/cycle DVE instruction."""
    name = "ABS_SUB_MIN_RED_ANT"
    for o in dve_ops.OPS:
        if o.name == name:
            return o

    def _ref(in0, in1, s0, s1, imm2):
        b = np.abs(in0.astype(np.float32) - s0).astype(np.float32)
        acc = np.minimum(
            np.float32(s1) if np.isscalar(s1) else s1.astype(np.float32),
            b.reshape(b.shape[0], -1).min(axis=-1, keepdims=True),
        )
        return b, acc

    spec = Spec(
        body=maxx(Src0 - C0, C0 - Src0),
        accum=minn,
        accum_init=C1,
        reference=_ref,
    )
    op = dve_ops.DveOp(name, spec, subdim=False, uops_sha={})
    row = dve_ops._CUSTOM_DVE_ROW_BASE + len(dve_ops.OPS)
    assert row < 0x20
    dve_ops.OPS.append(op)
    dve_ops.CUSTOM_DVE_SPECS[name] = spec
    dve_ops._SUB_OPCODE_FOR_NAME[name] = row
    for ver in ("v3", "v4"):
        compiled = DveOpSpec(
            name=name, opcode=row, uops=lower(spec, ver=ver),
            rd1_en=_has_src1(spec),
        )
        op.uops_sha[ver] = compiled.sha(ver)
    return op


ABSDIFF_MIN = _register_absdiff_min_op()

F32 = mybir.dt.float32
EPS = 1e-8

B, H, W = 4, 64, 64
N = H * W              # 4096 points per batch
HALF_ROWS = 32         # image rows per core
NI = HALF_ROWS * W     # 2048 gradient points per core
NTILES = NI // 128     # 16 i-tiles per core
NBLK = N // 128        # 32 column blocks of the colmin accumulator

def build_nc():
    nc = bacc.Bacc("TRN2", target_bir_lowering=False, debug=False)

    x_dram = nc.dram_tensor("xsh", [W, 3 * (HALF_ROWS + 2)], F32, kind="ExternalInput")
    b_dram = nc.dram_tensor("bvec", [N], F32, kind="ExternalInput")
    rowsum_dram = nc.dram_tensor("rowsum", [128, 1], F32, kind="ExternalOutput")
    colmin_dram = nc.dram_tensor("colmin", [128, NBLK], F32, kind="ExternalOutput")

    with tile.TileContext(nc) as tc:
        with (
            tc.tile_pool(name="consts", bufs=1) as consts,
            tc.tile_pool(name="sobel", bufs=1) as sobel,
            tc.tile_pool(name="bigbuf", bufs=1) as bigbuf,
            tc.tile_pool(name="dpool", bufs=3) as dpool,
            tc.tile_pool(name="dpoolB", bufs=3) as dpoolB,
            tc.tile_pool(name="psum_big", bufs=1, space="PSUM") as psum_big,
            tc.tile_pool(name="outs", bufs=1) as outs,
        ):
            # ---- Sobel, transposed layout (image cols on partitions). The
            # host supplies three column-shifted copies of the padded slab
            # (xm1 | x0 | xp1) so no cross-partition shifts are needed;
            # vertical taps are free-axis shifts.
            RP = HALF_ROWS + 2
            xsh = sobel.tile([W, 3 * RP], F32)
            nc.sync.dma_start(out=xsh[:], in_=x_dram.ap())
            b_row = bigbuf.tile([1, N], F32)
            nc.sync.dma_start(out=b_row[:], in_=b_dram.ap().unsqueeze(0))
            ones = consts.tile([1, 128], F32)
            nc.vector.memset(ones[:], 1.0)
            ps_big = psum_big.tile([128, N], F32)
            xm1, x0, xp1 = xsh[:, 0:RP], xsh[:, RP:2 * RP], xsh[:, 2 * RP:3 * RP]

            hd = sobel.tile([W, RP], F32)              # x[c-1] - x[c+1]
            nc.vector.tensor_tensor(hd[:], xm1, xp1, op=mybir.AluOpType.subtract)
            t1 = sobel.tile([W, RP], F32)
            nc.vector.tensor_add(t1[:], xm1, x0)
            t2 = sobel.tile([W, RP], F32)
            nc.vector.tensor_add(t2[:], x0, xp1)
            hs = sobel.tile([W, RP], F32)              # x[c-1] + 2x[c] + x[c+1]
            nc.vector.tensor_add(hs[:], t1[:], t2[:])

            # gx = vertical [1,2,1] on hd;  gy = vertical [1,0,-1] on hs
            pg = sobel.tile([W, HALF_ROWS + 1], F32)
            nc.vector.tensor_add(pg[:], hd[:, 0:HALF_ROWS + 1], hd[:, 1:HALF_ROWS + 2])
            gx = sobel.tile([W, HALF_ROWS], F32)
            nc.vector.tensor_add(gx[:], pg[:, 0:HALF_ROWS], pg[:, 1:HALF_ROWS + 1])
            gy = sobel.tile([W, HALF_ROWS], F32)
            nc.vector.tensor_tensor(
                gy[:], hs[:, 0:HALF_ROWS], hs[:, 2:HALF_ROWS + 2],
                op=mybir.AluOpType.subtract,
            )

            gx2 = sobel.tile([W, HALF_ROWS], F32)
            nc.vector.tensor_tensor(gx2[:], gx[:], gx[:], op=mybir.AluOpType.mult)
            gy2 = sobel.tile([W, HALF_ROWS], F32)
            nc.vector.tensor_tensor(gy2[:], gy[:], gy[:], op=mybir.AluOpType.mult)
            ssum = sobel.tile([W, HALF_ROWS], F32)
            nc.vector.scalar_tensor_tensor(
                ssum[:], gx2[:], EPS, gy2[:],
                op0=mybir.AluOpType.add, op1=mybir.AluOpType.add,
            )
            gT = sobel.tile([W, HALF_ROWS], F32)
            nc.scalar.activation(
                gT[:], ssum[:], mybir.ActivationFunctionType.Sqrt, bias=0.0
            )

            # g_s (128, 16): i-tile t = image rows {t, t+16};
            # partition p<64 -> (row t, col p); p>=64 -> (row t+16, col p-64)
            g_s = consts.tile([128, NTILES], F32)
            nc.vector.tensor_copy(g_s[0:64, :], gT[:, 0:NTILES])
            nc.vector.tensor_copy(g_s[64:128, :], gT[:, NTILES:2 * NTILES])

            # g broadcast for pass B: flatten gT to one partition (DMA),
            # then rank-1 PE broadcast to (128, 2048), like b_bcast.
            gflat = bigbuf.tile([1, NI], F32)
            nc.sync.dma_start(out=gflat[:], in_=gT[:])
            g_bcast = bigbuf.tile([128, NI], F32)
            for u in range(4):
                nc.tensor.matmul(
                    ps_big[:, u * 512:(u + 1) * 512], ones[:],
                    gflat[:, u * 512:(u + 1) * 512], start=True, stop=True,
                )
            nc.scalar.copy(g_bcast[:], ps_big[:, 0:NI])

            # b per-partition scalars for pass B: b_s[p, u] = b[p*32 + u]
            b_s = consts.tile([128, NBLK], F32)
            nc.sync.dma_start(
                out=b_s[:], in_=b_dram.ap().rearrange("(p u) -> p u", p=128)
            )

            # boundary broadcast (128, 4096) via rank-1 PE matmul (ones @ b);
            # stays resident in PSUM for the whole of pass A - ACT and the
            # custom DVE op read it there, skipping the SBUF copy.
            for u in range(8):
                nc.tensor.matmul(
                    ps_big[:, u * 512:(u + 1) * 512], ones[:],
                    b_row[:, u * 512:(u + 1) * 512], start=True, stop=True,
                )
            b_bcast = ps_big

            # ---- the two min passes. Per tile, one of two engine paths:
            #  - fused: one custom DVE op does |diff| + min-reduce (DVE only)
            #  - split: ACT produces |diff|, DVE reduces.
            # Order: fused pass-B tiles first (only need the g chain, so DVE
            # works while b_bcast builds), then pass A, then split pass-B.
            A_FUSED = 8
            B_FUSED = 16
            BIG = 3.0e38
            junk = bigbuf.tile([128, N], F32)

            rowmin_s = outs.tile([128, NTILES], F32)
            colmin_s = outs.tile([128, NBLK], F32)

            for u in range(B_FUSED):
                nc.vector._custom_dve(
                    ABSDIFF_MIN, out=junk[:, 0:NI],
                    accum_out=colmin_s[:, u:u + 1],
                    in0=g_bcast[:], s0=b_s[:, u:u + 1], s1=BIG,
                )

            for t in range(NTILES):
                if t < A_FUSED:
                    nc.vector._custom_dve(
                        ABSDIFF_MIN, out=junk[:],
                        accum_out=rowmin_s[:, t:t + 1],
                        in0=b_bcast[:], s0=g_s[:, t:t + 1], s1=BIG,
                    )
                else:
                    d = dpool.tile([128, N], F32)
                    nc.scalar.activation(
                        d[:], b_bcast[:], mybir.ActivationFunctionType.Abs,
                        bias=g_s[:, t:t + 1], scale=-1.0,
                    )
                    nc.vector.tensor_reduce(
                        rowmin_s[:, t:t + 1], d[:],
                        axis=mybir.AxisListType.X, op=mybir.AluOpType.min,
                    )

            for u in range(B_FUSED, NBLK):
                dB = dpoolB.tile([128, NI], F32)
                nc.scalar.activation(
                    dB[:], g_bcast[:], mybir.ActivationFunctionType.Abs,
                    bias=b_s[:, u:u + 1], scale=-1.0,
                )
                nc.vector.tensor_reduce(
                    colmin_s[:, u:u + 1], dB[:],
                    axis=mybir.AxisListType.X, op=mybir.AluOpType.min,
                )

            # ---- outputs
            rsum = outs.tile([128, 1], F32)
            nc.vector.tensor_reduce(
                rsum[:], rowmin_s[:], axis=mybir.AxisListType.X,
                op=mybir.AluOpType.add,
            )
            nc.sync.dma_start(out=rowsum_dram.ap(), in_=rsum[:])
            nc.sync.dma_start(out=colmin_dram.ap(), in_=colmin_s[:])

    nc.compile()
    return nc


_NC = None


def _get_nc():
    global _NC
    if _NC is None:
        _NC = build_nc()
    return _NC


def make_in_maps(depth_pred: np.ndarray, boundary_gt: np.ndarray):
    depth = np.asarray(depth_pred, np.float32).reshape(B, H, W)
    bnd = np.asarray(boundary_gt, np.float32).reshape(B, N)
    in_maps = []
    for k in range(8):
        bi, h = k // 2, k % 2
        r0 = h * HALF_ROWS
        slab = np.zeros((HALF_ROWS + 2, W), np.float32)  # rows r0-1 .. r0+32
        lo, hi = max(r0 - 1, 0), min(r0 + HALF_ROWS + 1, H)
        slab[lo - (r0 - 1):hi - (r0 - 1), :] = depth[bi, lo:hi, :]
        # three column-shifted copies: xsh[c] = [slab[:,c-1], slab[:,c], slab[:,c+1]]
        xsh = np.zeros((W, 3, HALF_ROWS + 2), np.float32)
        xsh[1:, 0, :] = slab[:, 0:W - 1].T
        xsh[:, 1, :] = slab.T
        xsh[0:W - 1, 2, :] = slab[:, 1:W].T
        in_maps.append({
            "xsh": np.ascontiguousarray(xsh.reshape(W, 3 * (HALF_ROWS + 2))),
            "bvec": np.ascontiguousarray(bnd[bi]),
        })
    return in_maps


def combine(results):
    dist1 = 0.0
    dist2 = 0.0
    for bi in range(B):
        dist1 += float(results[2 * bi]["rowsum"].sum(dtype=np.float64))
        dist1 += float(results[2 * bi + 1]["rowsum"].sum(dtype=np.float64))
        cm = np.minimum(results[2 * bi]["colmin"], results[2 * bi + 1]["colmin"])
        dist2 += float(cm.sum(dtype=np.float64))
    return np.float32(dist1 / (B * N) + dist2 / (B * N))


def kernel(depth_pred: np.ndarray, boundary_gt: np.ndarray) -> np.ndarray:
    nc = _get_nc()
    in_maps = make_in_maps(depth_pred, boundary_gt)
    res = run_bass_kernel_spmd(nc, in_maps, core_ids=list(range(8)))
    return combine(res.results)
